# revision 1
# baseline (speedup 1.0000x reference)
"""Bass/Trainium2 kernel for nn_BatchSeparationLoss.

reference:
    h = minmax-normalize(heatmaps) per (b, n) over spatial dims
    gram[b, i, j] = sum_hw h_i h_j
    out = sum of strict-lower-triangle of gram over all b / B

Algebraic reformulation (avoids materializing normalized tensor):
    inv_i = 1 / (max_i - min_i + eps)
    <h_i, h_j> = inv_i inv_j (G_ij - mn_i S_j - mn_j S_i + P mn_i mn_j)
  where G = X X^T (raw gram), S_i = sum(x_i), P = H*W.

Sharding: data-parallel over batch, 2 images per core (8 cores).
Each core emits one fp32 partial; host sums and divides by B.
"""

import os
import sys

import numpy as np

_REPO = "/opt/trn_rl_repo"
if _REPO not in sys.path:
    sys.path.insert(0, _REPO)

EPS = 1e-8
B, N, H, W = 16, 16, 224, 224
PIX = H * W          # 50176
CORES = 8
BPC = B // CORES     # 2 images per core
CH = BPC * N         # 32 channel rows per core
Q = 128              # SBUF partitions (spatial outer)
T = PIX // Q         # 392 spatial inner
K = 4                # split of T so matmul lhsT free dim = K*CH = 128
U = T // K           # 98 accumulation steps
VC = K * CH          # 128 virtual channels

_cache = {}


def _build():
    """Build the per-core Bass program (SPMD: same program, different shard)."""
    from concourse import bass, bacc, mybir
    from concourse.bass import MemorySpace
    from concourse.tile import TileContext

    f32 = mybir.dt.float32
    bf16 = mybir.dt.bfloat16
    Alu = mybir.AluOpType
    Act = mybir.ActivationFunctionType

    # Bacc (not plain Bass): its compile() pass splits multi-semaphore waits
    # into event-semaphore chains (TRN2 allows 1 wait per instruction).
    nc = bacc.Bacc(None)
    x = nc.declare_dram_parameter("x", [CH, PIX], f32, isOutput=False)
    mask = nc.declare_dram_parameter("mask", [CH, CH], f32, isOutput=False)
    ident = nc.declare_dram_parameter("ident", [128, 128], f32, isOutput=False)
    out = nc.declare_dram_parameter("out", [1, 1], f32, isOutput=True)

    with TileContext(nc) as tc:
        with (
            tc.tile_pool(name="main", bufs=1) as pool,
            tc.tile_pool(name="psum", bufs=1, space=MemorySpace.PSUM) as psum,
        ):
            X = pool.tile([Q, CH, T], f32)          # raw shard, 50 KB/partition
            Xb = pool.tile([Q, K, CH, U], bf16)     # bf16, (k,g) order, 25 KB/part
            idt = pool.tile([128, 128], f32)
            msk = pool.tile([CH, CH], f32)
            stats = pool.tile([Q, 3 * CH], f32)     # min | max | S partials

            # ---- load / cast / stats, pipelined in channel chunks ----
            # DMA chunks along g keep 1568 B contiguous runs; stats and cast
            # for chunk i overlap the DMA of chunk i+1, all at full 128-lane
            # width (q-chunking wasted 3/4 of the DVE lanes).
            x_v = x[:, :].rearrange("g (q t) -> q g t", q=Q)   # [128, 32, 392]
            nc.sync.dma_start(out=idt[:, :], in_=ident[:, :])
            nc.sync.dma_start(out=msk[:, :], in_=mask[:, :])
            # Two small leading chunks prime the DVE pipeline earlier (DVE
            # min+max is the longest engine total and starts after DMA 0).
            CHUNKS = [2, 2] + [4] * 6 + [2, 2]
            idtD = pool.tile([128, 128], f32)
            psMin = psum.tile([CH, Q], f32)
            psMax = psum.tile([CH, Q], f32)
            psSum = psum.tile([CH, Q], f32)
            mnC = pool.tile([CH, 1], f32)
            mxC = pool.tile([CH, 1], f32)
            SC = pool.tile([CH, 1], f32)
            rngC = pool.tile([CH, 1], f32)
            invC = pool.tile([CH, 1], f32)
            packA = pool.tile([CH, 32], f32)
            packB = pool.tile([CH, 32], f32)
            packV = pool.tile([CH, 32], f32)
            tA = pool.tile([CH, 32], f32)
            tB = pool.tile([CH, 32], f32)
            tV = pool.tile([CH, 32], f32)
            psumW = psum.tile([CH, CH], f32)
            wm = pool.tile([CH, CH], f32)

            nc.vector.memset(packA[:, :], 0.0)
            nc.vector.memset(packB[:, :], 0.0)
            nc.vector.memset(packV[:, :], 0.0)
            gs = 0
            for gc in CHUNKS:
                ge = gs + gc
                nc.sync.dma_start(out=X[:, gs:ge, :], in_=x_v[:, gs:ge, :])
                nc.vector.tensor_reduce(
                    out=stats[:, gs:ge], in_=X[:, gs:ge, :],
                    axis=mybir.AxisListType.X, op=Alu.min,
                )
                nc.vector.tensor_reduce(
                    out=stats[:, CH + gs:CH + ge], in_=X[:, gs:ge, :],
                    axis=mybir.AxisListType.X, op=Alu.max,
                )
                # cast per channel with fused running sum: the bf16 cast is an
                # ACT Copy, and accum_out gives S for free (no DVE pass)
                for g in range(gs, ge):
                    nc.scalar.activation(
                        out=Xb[:, :, g, :],
                        in_=X[:, g, :].rearrange("q (k u) -> q k u", k=K),
                        func=Act.Copy,
                        accum_out=stats[:, 2 * CH + g:2 * CH + g + 1],
                    )
                gs = ge

            def _emit_epi():
                # collapse partition axis: transpose [128, 32] -> [32, 128]
                nc.tensor.transpose(out=psMin[:, :], in_=stats[:, 0:CH], identity=idt[:, :])
                nc.tensor.transpose(out=psMax[:, :], in_=stats[:, CH:2 * CH], identity=idt[:, :])
                nc.tensor.transpose(out=psSum[:, :], in_=stats[:, 2 * CH:3 * CH], identity=idt[:, :])
                nc.vector.tensor_reduce(out=mnC[:, :], in_=psMin[:, :], axis=mybir.AxisListType.X, op=Alu.min)
                nc.vector.tensor_reduce(out=mxC[:, :], in_=psMax[:, :], axis=mybir.AxisListType.X, op=Alu.max)
                nc.vector.tensor_reduce(out=SC[:, :], in_=psSum[:, :], axis=mybir.AxisListType.X, op=Alu.add)
                nc.vector.scalar_tensor_tensor(
                    out=rngC[:, :], in0=mxC[:, :], scalar=float(EPS), in1=mnC[:, :],
                    op0=Alu.add, op1=Alu.subtract,
                )
                nc.vector.reciprocal(out=invC[:, :], in_=rngC[:, :])
                nc.vector.tensor_copy(packA[:, 0:1], mnC[:, :])
                nc.vector.scalar_tensor_tensor(
                    out=packA[:, 1:2], in0=mnC[:, :], scalar=float(PIX),
                    in1=SC[:, :], op0=Alu.mult, op1=Alu.subtract,
                )
                nc.vector.tensor_scalar_mul(packB[:, 0:1], SC[:, :], -1.0)
                nc.vector.tensor_copy(packB[:, 1:2], mnC[:, :])
                nc.vector.tensor_copy(packV[:, 0:1], invC[:, :])
                nc.vector.transpose(out=tA[:, :], in_=packA[:, :])
                nc.vector.transpose(out=tB[:, :], in_=packB[:, :])
                nc.vector.transpose(out=tV[:, :], in_=packV[:, :])
                nc.tensor.matmul(psumW[:, :], tV[0:1, 0:CH], tV[0:1, 0:CH], start=True, stop=True)
                # w = inv_i inv_j * mask (ready while the gram stream still runs)
                nc.vector.tensor_tensor(
                    out=wm[:, :], in0=psumW[:, :], in1=msk[:, :], op=Alu.mult
                )

            psumG = psum.tile([CH, CH], f32)
            _mm = 0
            for u in range(U):
                for k in range(K):
                    ap = Xb[:, k, :, u]
                    nc.tensor.matmul(
                        psumG[:, :], ap, ap, start=(_mm == 0),
                        stop=False, skip_group_check=True,
                    )
                    _mm += 1
                    if _mm == 235:
                        _emit_epi()
            nc.tensor.matmul(psumG[:, :], tA[0:2, 0:CH], tB[0:2, 0:CH],
                             start=False, stop=True, skip_group_check=True)

            # multiply then reduce (tensor_tensor_reduce crashes the device,
            # keep unfused); the /B scale is folded into the ones vector
            scr = pool.tile([CH, CH], f32)
            tot = pool.tile([CH, 1], f32)
            nc.vector.tensor_tensor(
                out=scr[:, :], in0=psumG[:, :], in1=wm[:, :], op=Alu.mult
            )
            nc.vector.tensor_reduce(
                out=tot[:, :], in_=scr[:, :], axis=mybir.AxisListType.X, op=Alu.add
            )
            ones32 = pool.tile([CH, 1], f32)
            nc.vector.memset(ones32[:, :], 1.0 / float(B))
            psumF = psum.tile([1, 1], f32)
            nc.tensor.matmul(psumF[:, :], ones32[:, :], tot[:, :],
                             start=True, stop=True)
            res = pool.tile([1, 1], f32)
            nc.vector.tensor_copy(res[:, :], psumF[:, :])
            nc.sync.dma_start(out=out[0:1, 0:1], in_=res[0:1, 0:1])

    nc.finalize()
    return nc


def _mask_np():
    m = np.zeros((CH, CH), np.float32)
    for b in range(BPC):
        m[16 * b:16 * b + 16, 16 * b:16 * b + 16] = np.tril(
            np.ones((16, 16), np.float32), k=-1
        )
    return m


def kernel(heatmaps: np.ndarray) -> np.ndarray:
    from concourse.bass_utils import run_bass_kernel_spmd

    if "nc" not in _cache:
        _cache["nc"] = _build()
    nc = _cache["nc"]

    hm = np.ascontiguousarray(np.asarray(heatmaps, dtype=np.float32))
    mask = _mask_np()
    ident = np.eye(128, dtype=np.float32)
    in_maps = []
    for c in range(CORES):
        shard = hm[c * BPC:(c + 1) * BPC].reshape(CH, PIX)
        in_maps.append({"x": shard, "mask": mask, "ident": ident})

    res = run_bass_kernel_spmd(nc, in_maps, list(range(CORES))).results
    total = sum(float(r["out"][0, 0]) for r in res)
    return np.array(total, dtype=np.float32)



# revision 28
# speedup vs baseline: 1.5987x; 1.5987x over previous
"""Bass/Trainium2 kernel for nn_BatchSeparationLoss.

reference:
    h = minmax-normalize(heatmaps) per (b, n) over spatial dims
    gram[b, i, j] = sum_hw h_i h_j
    out = sum of strict-lower-triangle of gram over all b / B

Algebraic reformulation (avoids materializing the normalized tensor):
    inv_i = 1 / (max_i - min_i + eps)
    <h_i, h_j> = inv_i inv_j (G_ij - mn_i S_j - mn_j S_i + P mn_i mn_j)
  where G = X X^T (raw gram), S_i = sum(x_i), P = H*W.

Sharding: data-parallel over batch, 2 images per core (8 cores); the host
sums the per-core [32, 32] weighted-gram outputs and divides by B.

v4 design (validated against the neuronxcc backend verifier):
  - Host lays the shard out pixel-major [128 part, 392 t, 32 ch] so chunk
    DMAs have multi-KB contiguous runs per partition.
  - All chunk loads are Pool-engine SWDGE DMAs casting fp32->fp16 in
    flight (halves SBUF traffic; verified legal on HW).
  - Gram + per-channel sums accumulate on the PE as per-column matmuls
    (cost is per output row, so the 128-deep contraction is free; sums
    ride along as 1-row matmuls against a ones vector).
  - Per-channel min/max: running fp16 tensor_tensor chains on the DVE
    (2x perf mode), one fold per chunk per tree, then short tail trees.
  - Cross-partition stats via GpSimd cross-lane (C axis) max reduces
    (min side negated first - cross-lane min is not supported).
  - Epilogue: two rank-1 matmuls fold the mean/min terms into the gram
    psum; inv outer product weights it; mask applied; [32, 32] shipped.
"""

import sys

import numpy as np

_REPO = "/opt/trn_rl_repo"
if _REPO not in sys.path:
    sys.path.insert(0, _REPO)

EPS = 1e-8
B, N, H, W = 16, 16, 224, 224
PIX = H * W          # 50176
CORES = 8
BPC = B // CORES     # 2 images per core
CH = BPC * N         # 32 channel rows per core
Q = 128              # SBUF partitions
T = PIX // Q         # 392 t-columns per partition
TW = 49              # chunk width (t columns)
NCHUNK = T // TW     # 8 chunks

# chunk load/processing order (all chunks via Pool cast-DMA)
ORDER = [0, 2, 4, 6, 1, 3, 5, 7]

_cache = {}


def _build():
    from concourse import bass, bacc, mybir
    from concourse.bass import MemorySpace
    from concourse.tile import TileContext

    f32 = mybir.dt.float32
    f16 = mybir.dt.float16
    Alu = mybir.AluOpType
    Act = mybir.ActivationFunctionType

    nc = bacc.Bacc(None)
    x = nc.declare_dram_parameter("x", [Q, T * CH], f32, isOutput=False)
    mask = nc.declare_dram_parameter("mask", [CH, CH], f32, isOutput=False)
    idt32 = nc.declare_dram_parameter("idt32", [CH, CH], f32, isOutput=False)
    out = nc.declare_dram_parameter("out", [CH, CH], f32, isOutput=True)

    xv = x[:, :].rearrange("q (t g) -> q t g", g=CH)

    with TileContext(nc) as tc:
        with (
            tc.tile_pool(name="main", bufs=1) as pool,
            tc.tile_pool(name="psum", bufs=1, space=MemorySpace.PSUM) as psum,
        ):
            X16 = pool.tile([Q, T, CH], f16)
            ones16 = pool.tile([Q, 1], f16)
            idt32_t = pool.tile([CH, CH], f32)
            msk = pool.tile([CH, CH], f32)
            warm = pool.tile([1, 1], f32)

            nc.vector.memset(ones16[:, :], 1.0)
            nc.vector.memset(warm[:, :], 1.0)
            # absorb the Act engine's activation-table load while idle
            nc.scalar.copy(warm[:, :], warm[:, :])

            def sl(c):
                return slice(TW * c, TW * (c + 1))

            # ---- loads: Pool SWDGE cast-DMAs; small constants on SP ----
            for c in ORDER:
                nc.gpsimd.dma_start(out=X16[:, sl(c), :], in_=xv[:, sl(c), :])
            nc.sync.dma_start(out=idt32_t[:, :], in_=idt32[:, :])
            nc.sync.dma_start(out=msk[:, :], in_=mask[:, :])

            # ---- PE: gram + sums, per column, chunks in load order ----
            G = psum.tile([CH, CH], f32)
            S = psum.tile([CH, 1], f32)
            first = True
            for c in ORDER:
                for t in range(TW * c, TW * (c + 1)):
                    col = X16[:, t, :]
                    nc.tensor.matmul(G[:, :], col, col, start=first,
                                     stop=False, skip_group_check=True)
                    nc.tensor.matmul(S[:, :], col, ones16[:, :], start=first,
                                     stop=(t == TW * c + TW - 1 and c == ORDER[-1]),
                                     skip_group_check=True)
                    first = False

            # ---- DVE: min chain first (its tail + stats then hide inside
            # the max chain), max chain after ----
            Rmin = pool.tile([Q, TW, CH], f16)
            Rmax = pool.tile([Q, TW, CH], f16)
            c0, c1 = ORDER[0], ORDER[1]
            nc.vector.tensor_tensor(out=Rmin[:, :, :], in0=X16[:, sl(c0), :],
                                    in1=X16[:, sl(c1), :], op=Alu.min)
            for c in ORDER[2:]:
                nc.vector.tensor_tensor(out=Rmin[:, :, :], in0=Rmin[:, :, :],
                                        in1=X16[:, sl(c), :], op=Alu.min)
            # ---- tail trees 49 -> 1 ----
            tailn = [0]

            def tail(R, op, dst):
                tailn[0] += 1
                k = tailn[0]
                t1 = pool.tile([Q, 24, CH], f16, name=f"tl1_{k}")
                nc.vector.tensor_tensor(out=t1[:, :, :], in0=R[:, 0:24, :],
                                        in1=R[:, 24:48, :], op=op)
                t2 = pool.tile([Q, 12, CH], f16, name=f"tl2_{k}")
                nc.vector.tensor_tensor(out=t2[:, :, :], in0=t1[:, 0:12, :],
                                        in1=t1[:, 12:24, :], op=op)
                t3 = pool.tile([Q, 6, CH], f16, name=f"tl3_{k}")
                nc.vector.tensor_tensor(out=t3[:, :, :], in0=t2[:, 0:6, :],
                                        in1=t2[:, 6:12, :], op=op)
                t4 = pool.tile([Q, 3, CH], f16, name=f"tl4_{k}")
                nc.vector.tensor_tensor(out=t4[:, :, :], in0=t3[:, 0:3, :],
                                        in1=t3[:, 3:6, :], op=op)
                t5 = pool.tile([Q, 1, CH], f16, name=f"tl5_{k}")
                nc.vector.tensor_tensor(out=t5[:, :, :], in0=t4[:, 0:1, :],
                                        in1=t4[:, 1:2, :], op=op)
                nc.vector.tensor_tensor(out=t5[:, :, :], in0=t5[:, :, :],
                                        in1=t4[:, 2:3, :], op=op)
                nc.vector.tensor_tensor(out=dst, in0=t5[:, :, :],
                                        in1=R[:, 48:49, :], op=op)

            # ---- min side completes early: tail, stats, corrections ----
            smin = pool.tile([Q, 1, CH], f16)
            smax = pool.tile([Q, 1, CH], f16)
            tail(Rmin, Alu.min, smin[:, :, :])
            nsmin = pool.tile([Q, CH], f16)
            nc.vector.tensor_scalar_mul(nsmin[:, :], smin[:, 0, :], -1.0)
            Mneg = pool.tile([1, CH], f32)   # = -mn row (cross-lane max)
            nc.gpsimd.tensor_reduce(out=Mneg[:, :], in_=nsmin[:, :],
                                    axis=mybir.AxisListType.C, op=Alu.max)
            mnR = pool.tile([1, CH], f32)
            nc.scalar.mul(mnR[:, :], Mneg[:, :], -1.0)
            # S as a row via PE transpose
            SC = pool.tile([CH, 1], f32)
            nc.scalar.copy(SC[:, :], S[:, :])
            pSr = psum.tile([1, CH], f32)
            nc.tensor.transpose(out=pSr[:, :], in_=SC[:, :], identity=idt32_t[:, :])
            nSr = pool.tile([1, CH], f32)
            nc.scalar.mul(nSr[:, :], pSr[:, :], -1.0)
            vR = pool.tile([1, CH], f32)
            nc.vector.scalar_tensor_tensor(
                out=vR[:, :], in0=mnR[:, :], scalar=float(PIX), in1=pSr[:, :],
                op0=Alu.mult, op1=Alu.subtract)
            # rank-1 corrections close the G group
            nc.tensor.matmul(G[:, :], mnR[:, :], vR[:, :],
                             start=False, stop=False, skip_group_check=True)
            nc.tensor.matmul(G[:, :], nSr[:, :], mnR[:, :],
                             start=False, stop=True, skip_group_check=True)
            masked = pool.tile([CH, CH], f32)
            nc.vector.tensor_tensor(out=masked[:, :], in0=G[:, :], in1=msk[:, :],
                                    op=Alu.mult)

            # ---- max chain (late path); the last folds are floored so the
            # min tail + stats can claim the DVE first ----
            nc.vector.tensor_tensor(out=Rmax[:, :, :], in0=X16[:, sl(c0), :],
                                    in1=X16[:, sl(c1), :], op=Alu.max)
            for i, c in enumerate(ORDER[2:]):
                with tc.tile_wait_until(0.0145, enable=(i >= 3)):
                    nc.vector.tensor_tensor(out=Rmax[:, :, :], in0=Rmax[:, :, :],
                                            in1=X16[:, sl(c), :], op=Alu.max)
            tail(Rmax, Alu.max, smax[:, :, :])
            mxR = pool.tile([1, CH], f32)
            nc.gpsimd.tensor_reduce(out=mxR[:, :], in_=smax[:, 0, :],
                                    axis=mybir.AxisListType.C, op=Alu.max)

            # inverse-range weights
            rngR = pool.tile([1, CH], f32)
            nc.vector.scalar_tensor_tensor(
                out=rngR[:, :], in0=mxR[:, :], scalar=float(EPS), in1=mnR[:, :],
                op0=Alu.add, op1=Alu.subtract)
            invR = pool.tile([1, CH], f32)
            nc.vector.reciprocal(out=invR[:, :], in_=rngR[:, :])
            pW = psum.tile([CH, CH], f32)
            nc.tensor.matmul(pW[:, :], invR[:, :], invR[:, :],
                             start=True, stop=True, skip_group_check=True)
            scr2 = pool.tile([CH, CH], f32)
            nc.vector.tensor_tensor(out=scr2[:, :], in0=masked[:, :],
                                    in1=pW[:, :], op=Alu.mult)
            nc.sync.dma_start(out=out[:, :], in_=scr2[:, :])

    nc.finalize()
    return nc


def _mask_np():
    m = np.zeros((CH, CH), np.float32)
    for b in range(BPC):
        m[16 * b:16 * b + 16, 16 * b:16 * b + 16] = np.tril(
            np.ones((16, 16), np.float32), k=-1)
    return m


def _shard_np(hm, c):
    s = hm[c * BPC:(c + 1) * BPC].reshape(CH, Q, T)
    return np.ascontiguousarray(s.transpose(1, 2, 0)).reshape(Q, T * CH)


def kernel(heatmaps: np.ndarray) -> np.ndarray:
    from concourse.bass_utils import run_bass_kernel_spmd

    if "nc" not in _cache:
        _cache["nc"] = _build()
    nc = _cache["nc"]

    hm = np.ascontiguousarray(np.asarray(heatmaps, dtype=np.float32))
    mask = _mask_np()
    i32 = np.eye(CH, dtype=np.float32)
    in_maps = []
    for c in range(CORES):
        in_maps.append({"x": _shard_np(hm, c), "mask": mask, "idt32": i32})

    res = run_bass_kernel_spmd(nc, in_maps, list(range(CORES))).results
    total = np.float32(sum(np.float32(r["out"].sum()) for r in res))
    return np.float32(total / np.float32(B))


# revision 36
# speedup vs baseline: 1.6799x; 1.0508x over previous
"""Bass/Trainium2 kernel for nn_BatchSeparationLoss.

reference:
    h = minmax-normalize(heatmaps) per (b, n) over spatial dims
    gram[b, i, j] = sum_hw h_i h_j
    out = sum of strict-lower-triangle of gram over all b / B

Algebraic reformulation (avoids materializing the normalized tensor):
    inv_i = 1 / (max_i - min_i + eps)
    <h_i, h_j> = inv_i inv_j (G_ij - mn_i S_j - mn_j S_i + P mn_i mn_j)
  where G = X X^T (raw gram), S_i = sum(x_i), P = H*W.

Sharding: data-parallel over batch, 2 images per core (8 cores); the host
sums the per-core [32, 32] weighted-gram outputs and divides by B.

v4 design (validated against the neuronxcc backend verifier):
  - Host lays the shard out pixel-major [128 part, 392 t, 32 ch] so chunk
    DMAs have multi-KB contiguous runs per partition.
  - All chunk loads are Pool-engine SWDGE DMAs casting fp32->fp16 in
    flight (halves SBUF traffic; verified legal on HW).
  - Gram + per-channel sums accumulate on the PE as per-column matmuls
    (cost is per output row, so the 128-deep contraction is free; sums
    ride along as 1-row matmuls against a ones vector).
  - Per-channel min/max: running fp16 tensor_tensor chains on the DVE
    (2x perf mode), one fold per chunk per tree, then short tail trees.
    The last max-chain folds carry tile_wait_until floors so the min
    side's tail/stats can claim the DVE as soon as its chain finishes.
  - Cross-partition stats via GpSimd cross-lane (C axis) max reduces
    (min side negated first - cross-lane min is not supported).
  - Epilogue: two rank-1 matmuls fold the mean/min terms into the gram
    psum; inv outer product weights it; mask applied; [32, 32] shipped.
"""

import sys

import numpy as np

_REPO = "/opt/trn_rl_repo"
if _REPO not in sys.path:
    sys.path.insert(0, _REPO)

EPS = 1e-8
B, N, H, W = 16, 16, 224, 224
PIX = H * W          # 50176
CORES = 8
BPC = B // CORES     # 2 images per core
CH = BPC * N         # 32 channel rows per core
Q = 128              # SBUF partitions
T = PIX // Q         # 392 t-columns per partition
TW = 28              # chunk width (t columns)
NCHUNK = T // TW     # 14 chunks

# chunk load/processing order (all chunks via Pool cast-DMA)
ORDER = list(range(14))

_cache = {}


def _build():
    from concourse import bass, bacc, mybir
    from concourse.bass import MemorySpace
    from concourse.tile import TileContext

    f32 = mybir.dt.float32
    f16 = mybir.dt.float16
    Alu = mybir.AluOpType

    nc = bacc.Bacc(None)
    x = nc.declare_dram_parameter("x", [Q, T * CH], f32, isOutput=False)
    idt32 = nc.declare_dram_parameter("idt32", [CH, CH], f32, isOutput=False)
    out = nc.declare_dram_parameter("out", [CH, CH], f32, isOutput=True)

    xv = x[:, :].rearrange("q (t g) -> q t g", g=CH)

    with TileContext(nc) as tc:
        with (
            tc.tile_pool(name="main", bufs=1) as pool,
            tc.tile_pool(name="psum", bufs=1, space=MemorySpace.PSUM) as psum,
        ):
            X16 = pool.tile([Q, T, CH], f16)
            ones16 = pool.tile([Q, 1], f16)
            idt32_t = pool.tile([CH, CH], f32)
            warm = pool.tile([1, 1], f32)

            nc.vector.memset(ones16[:, :], 1.0)
            nc.vector.memset(warm[:, :], 1.0)
            # absorb the Act engine's activation-table load while idle
            nc.scalar.copy(warm[:, :], warm[:, :])

            def sl(c):
                return slice(TW * c, TW * (c + 1))

            # ---- loads: Pool SWDGE cast-DMAs; small constants on SP ----
            for c in ORDER:
                nc.gpsimd.dma_start(out=X16[:, sl(c), :], in_=xv[:, sl(c), :])
            nc.sync.dma_start(out=idt32_t[:, :], in_=idt32[:, :])

            # ---- PE: gram + sums, per column, chunks in load order ----
            G = psum.tile([CH, CH], f32)
            S = psum.tile([CH, 1], f32)
            first = True
            for c in ORDER:
                for t in range(TW * c, TW * (c + 1)):
                    col = X16[:, t, :]
                    nc.tensor.matmul(G[:, :], col, col, start=first,
                                     stop=False, skip_group_check=True)
                    nc.tensor.matmul(S[:, :], col, ones16[:, :], start=first,
                                     stop=(t == TW * c + TW - 1 and c == ORDER[-1]),
                                     skip_group_check=True)
                    first = False

            # ---- DVE: min chain first (its tail + stats then hide inside
            # the max chain), max chain after ----
            Rmin = pool.tile([Q, TW, CH], f16)
            Rmax = pool.tile([Q, TW, CH], f16)
            c0, c1 = ORDER[0], ORDER[1]
            nc.vector.tensor_tensor(out=Rmin[:, :, :], in0=X16[:, sl(c0), :],
                                    in1=X16[:, sl(c1), :], op=Alu.min)
            for c in ORDER[2:]:
                nc.vector.tensor_tensor(out=Rmin[:, :, :], in0=Rmin[:, :, :],
                                        in1=X16[:, sl(c), :], op=Alu.min)
            # ---- tail trees 28 -> 1 ----
            tailn = [0]

            def tail(R, op, dst):
                tailn[0] += 1
                k = tailn[0]
                t1 = pool.tile([Q, 14, CH], f16, name=f"tl1_{k}")
                nc.vector.tensor_tensor(out=t1[:, :, :], in0=R[:, 0:14, :],
                                        in1=R[:, 14:28, :], op=op)
                t2 = pool.tile([Q, 7, CH], f16, name=f"tl2_{k}")
                nc.vector.tensor_tensor(out=t2[:, :, :], in0=t1[:, 0:7, :],
                                        in1=t1[:, 7:14, :], op=op)
                t3 = pool.tile([Q, 3, CH], f16, name=f"tl3_{k}")
                nc.vector.tensor_tensor(out=t3[:, :, :], in0=t2[:, 0:3, :],
                                        in1=t2[:, 3:6, :], op=op)
                t4 = pool.tile([Q, 1, CH], f16, name=f"tl4_{k}")
                nc.vector.tensor_tensor(out=t4[:, :, :], in0=t3[:, 0:1, :],
                                        in1=t3[:, 1:2, :], op=op)
                nc.vector.tensor_tensor(out=t4[:, :, :], in0=t4[:, :, :],
                                        in1=t3[:, 2:3, :], op=op)
                nc.vector.tensor_tensor(out=dst, in0=t4[:, :, :],
                                        in1=t2[:, 6:7, :], op=op)

            # ---- min side: tail entirely off the DVE ----
            # Act negates the folded chunk, Pool collapses partitions with a
            # cross-lane max, an SP DMA spreads the row across 28 partitions,
            # and a second cross-lane max finishes the t direction.
            smax = pool.tile([Q, 1, CH], f16)
            negmin = pool.tile([Q, TW, CH], f16)
            nc.scalar.mul(negmin[:, :, :], Rmin[:, :, :], -1.0)
            rowF = pool.tile([1, TW * CH], f32)
            nc.gpsimd.tensor_reduce(
                out=rowF[:, :].rearrange("o (t g) -> o t g", g=CH),
                in_=negmin[:, :, :],
                axis=mybir.AxisListType.C, op=Alu.max)
            sprMin = pool.tile([TW, CH], f32)
            nc.sync.dma_start(out=sprMin[:, :], in_=rowF[0:1, :])
            Mneg = pool.tile([1, CH], f32)   # = -mn row (cross-lane max)
            nc.gpsimd.tensor_reduce(out=Mneg[:, :], in_=sprMin[:, :],
                                    axis=mybir.AxisListType.C, op=Alu.max)
            mnR = pool.tile([1, CH], f32)
            nc.scalar.mul(mnR[:, :], Mneg[:, :], -1.0)
            # S as a row via PE transpose
            SC = pool.tile([CH, 1], f32)
            nc.scalar.copy(SC[:, :], S[:, :])
            pSr = psum.tile([1, CH], f32)
            nc.tensor.transpose(out=pSr[:, :], in_=SC[:, :], identity=idt32_t[:, :])
            nSr = pool.tile([1, CH], f32)
            nc.scalar.mul(nSr[:, :], pSr[:, :], -1.0)
            PmnR = pool.tile([1, CH], f32)
            nc.scalar.mul(PmnR[:, :], Mneg[:, :], -float(PIX))
            # rank-1 corrections close the G group:
            # G += P*mn (x) mn - mn (x) S - S (x) mn
            nc.tensor.matmul(G[:, :], PmnR[:, :], mnR[:, :],
                             start=False, stop=False, skip_group_check=True)
            nc.tensor.matmul(G[:, :], mnR[:, :], nSr[:, :],
                             start=False, stop=False, skip_group_check=True)
            nc.tensor.matmul(G[:, :], nSr[:, :], mnR[:, :],
                             start=False, stop=True, skip_group_check=True)
            # ---- max chain (late path); the last folds are floored so the
            # min tail + stats can claim the DVE first ----
            nc.vector.tensor_tensor(out=Rmax[:, :, :], in0=X16[:, sl(c0), :],
                                    in1=X16[:, sl(c1), :], op=Alu.max)
            for i, c in enumerate(ORDER[2:]):
                with tc.tile_wait_until(0.0124, enable=(i >= 4)):
                    nc.vector.tensor_tensor(out=Rmax[:, :, :], in0=Rmax[:, :, :],
                                            in1=X16[:, sl(c), :], op=Alu.max)
            tail(Rmax, Alu.max, smax[:, :, :])
            mxR = pool.tile([1, CH], f32)
            nc.gpsimd.tensor_reduce(out=mxR[:, :], in_=smax[:, 0, :],
                                    axis=mybir.AxisListType.C, op=Alu.max)

            # inverse-range weights
            rngR = pool.tile([1, CH], f32)
            nc.vector.scalar_tensor_tensor(
                out=rngR[:, :], in0=mxR[:, :], scalar=float(EPS), in1=mnR[:, :],
                op0=Alu.add, op1=Alu.subtract)
            invR = pool.tile([1, CH], f32)
            nc.vector.reciprocal(out=invR[:, :], in_=rngR[:, :])
            pW = psum.tile([CH, CH], f32)
            nc.tensor.matmul(pW[:, :], invR[:, :], invR[:, :],
                             start=True, stop=True, skip_group_check=True)
            Gsb = pool.tile([CH, CH], f32)
            nc.scalar.copy(Gsb[:, :], G[:, :])
            scr2 = pool.tile([CH, CH], f32)
            nc.vector.tensor_tensor(out=scr2[:, :], in0=Gsb[:, :],
                                    in1=pW[:, :], op=Alu.mult)
            nc.sync.dma_start(out=out[:, :], in_=scr2[:, :])

    nc.finalize()
    return nc


def _mask_np():
    m = np.zeros((CH, CH), np.float32)
    for b in range(BPC):
        m[16 * b:16 * b + 16, 16 * b:16 * b + 16] = np.tril(
            np.ones((16, 16), np.float32), k=-1)
    return m


def _shard_np(hm, c):
    s = hm[c * BPC:(c + 1) * BPC].reshape(CH, Q, T)
    return np.ascontiguousarray(s.transpose(1, 2, 0)).reshape(Q, T * CH)


def kernel(heatmaps: np.ndarray) -> np.ndarray:
    from concourse.bass_utils import run_bass_kernel_spmd

    if "nc" not in _cache:
        _cache["nc"] = _build()
    nc = _cache["nc"]

    hm = np.ascontiguousarray(np.asarray(heatmaps, dtype=np.float32))
    mask = _mask_np()
    i32 = np.eye(CH, dtype=np.float32)
    in_maps = []
    for c in range(CORES):
        in_maps.append({"x": _shard_np(hm, c), "idt32": i32})

    res = run_bass_kernel_spmd(nc, in_maps, list(range(CORES))).results
    total = np.float32(sum(np.float32((r["out"] * mask).sum()) for r in res))
    return np.float32(total / np.float32(B))
